# revision 4
# baseline (speedup 1.0000x reference)
"""GCN mix encoder (3-layer SpMM + batch gather) on 8 Trainium2 NeuronCores.

Strategy (row-sharded SpMM, slot-permuted activations):
  - Nodes (N=150k) are row-sharded across 8 cores (18750 rows each).
  - Per core, rows are bin-packed into blocks of <=128 rows with <=P_CH*128
    adjacency nnz. Each block's segment-sum is computed as a chain of
    one-hot matmuls on the PE: psum[rows, D] += S_c.T @ G_c, where G_c is a
    128-row indirect-DMA gather of source embeddings and
    S_c[k, r] = val_k * (local_row_k == r) is built by one fused DVE
    tensor_scalar (is_equal then mult) against an iota tile.
  - Layer outputs live in *slot order* (block*128 + lane). The AllGather
    replicates the slot-ordered shards; the next layer's gather indices are
    pre-mapped on the host from node ids to slot positions, so no scatter
    or reordering is ever needed on-device.
  - Layer 3 is truncated to the rows referenced by the users/items batch;
    the mean over {ego0..ego3} is computed by gathering rows of the three
    stored activations at those indices and adding the layer-3 result.

Host does only index routing/packing (numpy); all embedding math and data
movement of the layers runs on the NeuronCores.
"""

import numpy as np

import concourse.bass as bass
import concourse.bacc as bacc
import concourse.mybir as mybir
import concourse.tile as tile
from concourse.bass_utils import run_bass_kernel_spmd

N_CORES = 8
USER_COUNT = 100_000
ITEM_COUNT = 50_000
N_NODES = USER_COUNT + ITEM_COUNT
EMB = 128
N_LAYERS = 3
SHARD = N_NODES // N_CORES  # 18750
P = 128
P_CH_MIN = 11
SB_N = 4  # blocks per superblock (output DMA granularity)


def _bin_pack(items, weights, cap_w, cap_n=P):
    """Pack items (in order) into blocks with <=cap_n items, <=cap_w weight."""
    blocks, cur, cur_w = [], [], 0
    for it, w in zip(items, weights):
        w = int(w)
        if cur and (len(cur) >= cap_n or cur_w + w > cap_w):
            blocks.append(cur)
            cur, cur_w = [], 0
        cur.append(it)
        cur_w += w
    if cur:
        blocks.append(cur)
    return blocks


def _fill_slots(blocks, degs, row_start, cols_src, vals_src, p_ch, nblk):
    """Lay nnz into the [P, nblk*p_ch] slot grids.

    blocks: per-block list of row keys (indices into degs/row_start space)
    Returns cols (int32, natural col ids), lr (f32), val (f32).
    """
    nch = nblk * p_ch
    cols = np.zeros((P, nch), dtype=np.int64)
    lr = np.zeros((P, nch), dtype=np.float32)
    val = np.zeros((P, nch), dtype=np.float32)
    for b, rows in enumerate(blocks):
        out_i = 0
        for li, r in enumerate(rows):
            s, e = int(row_start[r]), int(row_start[r + 1])
            n = e - s
            if n == 0:
                continue
            sl = np.arange(out_i, out_i + n)
            ch = b * p_ch + sl // P
            lane = sl % P
            cols[lane, ch] = cols_src[s:e]
            lr[lane, ch] = li
            val[lane, ch] = vals_src[s:e]
            out_i += n
        assert out_i <= p_ch * P
    return cols, lr, val


def _build_nc(nblk, p_ch, nblk3, p_ch3):
    nch = nblk * p_ch
    nch3 = nblk3 * p_ch3
    nslot = nblk * P
    f32, i32 = mybir.dt.float32, mybir.dt.int32

    nc = bacc.Bacc("TRN2", target_bir_lowering=False, debug=False, num_devices=N_CORES)
    ego0 = nc.dram_tensor("ego0", [N_NODES, EMB], f32, kind="ExternalInput")
    ins = {}
    for name, shape, dt in [
        ("cols1", [P, nch], i32),
        ("cols2", [P, nch], i32),
        ("lr", [P, nch], f32),
        ("val", [P, nch], f32),
        ("cols3", [P, nch3], i32),
        ("lr3", [P, nch3], f32),
        ("val3", [P, nch3], f32),
        ("outrow_nat", [P, nblk3], i32),
        ("outrow_slot", [P, nblk3], i32),
        ("iota", [P, P], f32),
    ]:
        ins[name] = nc.dram_tensor(name, shape, dt, kind="ExternalInput")
    outbuf = nc.dram_tensor("outbuf", [nblk3 * P, EMB], f32, kind="ExternalOutput")

    with tile.TileContext(nc) as tc:
        with (
            tc.tile_pool(name="res", bufs=1) as res,
            tc.tile_pool(name="gp", bufs=6) as gp,
            tc.tile_pool(name="sp", bufs=6) as sp,
            tc.tile_pool(name="pp", bufs=4, space="PSUM") as pp,
            tc.tile_pool(name="st", bufs=2) as st,
            tc.tile_pool(name="dram", bufs=1, space="DRAM") as dram,
        ):
            sb = {}
            for name, t in ins.items():
                sb[name] = res.tile(list(t.shape), t.dtype, name=f"{name}_sb")
                nc.sync.dma_start(out=sb[name][:], in_=t[:, :])

            out_sb = res.tile([P, nblk * EMB], f32)

            ag_in = [dram.tile([nslot, EMB], f32, name=f"ag_in{t}") for t in range(2)]
            ego_full = [
                dram.tile(
                    [N_CORES * nslot, EMB], f32, name=f"ego_full{t}",
                    addr_space="Shared",
                )
                for t in range(2)
            ]

            def spmm_block(src_ap, b, p_ch_, cols_t, lr_t, val_t, dst_tile, dst_off):
                ps = pp.tile([P, EMB], f32, name="ps", tag="ps")
                for c in range(p_ch_):
                    j = b * p_ch_ + c
                    g = gp.tile([P, EMB], f32, name="g", tag="g")
                    nc.gpsimd.indirect_dma_start(
                        out=g[:],
                        out_offset=None,
                        in_=src_ap,
                        in_offset=bass.IndirectOffsetOnAxis(
                            ap=cols_t[:, j : j + 1], axis=0
                        ),
                    )
                    s = sp.tile([P, P], f32, name="s", tag="s")
                    nc.vector.tensor_scalar(
                        out=s[:],
                        in0=sb["iota"][:],
                        scalar1=lr_t[:, j : j + 1],
                        scalar2=val_t[:, j : j + 1],
                        op0=mybir.AluOpType.is_equal,
                        op1=mybir.AluOpType.mult,
                    )
                    nc.tensor.matmul(
                        ps[:], lhsT=s[:], rhs=g[:],
                        start=(c == 0), stop=(c == p_ch_ - 1),
                    )
                nc.scalar.copy(dst_tile[:, dst_off : dst_off + EMB], ps[:])

            # ---- layers 1..2 (full row shard, slot-ordered output) ----
            for t in range(2):
                src = ego0[:, :] if t == 0 else ego_full[0][:]
                cols_t = sb["cols1"] if t == 0 else sb["cols2"]
                for sb0 in range(0, nblk, SB_N):
                    nsb = min(SB_N, nblk - sb0)
                    for bi in range(nsb):
                        b = sb0 + bi
                        spmm_block(
                            src, b, p_ch, cols_t, sb["lr"], sb["val"],
                            out_sb, b * EMB,
                        )
                    nc.sync.dma_start(
                        out=ag_in[t][sb0 * P : (sb0 + nsb) * P, :].rearrange(
                            "(b p) d -> p b d", p=P
                        ),
                        in_=out_sb[:, sb0 * EMB : (sb0 + nsb) * EMB].rearrange(
                            "p (b d) -> p b d", d=EMB
                        ),
                    )
                nc.gpsimd.collective_compute(
                    "AllGather",
                    mybir.AluOpType.bypass,
                    replica_groups=[list(range(N_CORES))],
                    ins=[ag_in[t][:].opt()],
                    outs=[ego_full[t][:].opt()],
                )

            # ---- layer 3 (only output rows) ----
            l3stage = res.tile([P, nblk3 * EMB], f32)
            for b in range(nblk3):
                spmm_block(
                    ego_full[1][:], b, p_ch3, sb["cols3"], sb["lr3"], sb["val3"],
                    l3stage, b * EMB,
                )

            # ---- final mean: l3 + gathered ego0/ego1/ego2 rows ----
            acc = res.tile([P, nblk3 * EMB], f32)
            srcs = [
                (ego0[:, :], sb["outrow_nat"]),
                (ego_full[0][:], sb["outrow_slot"]),
                (ego_full[1][:], sb["outrow_slot"]),
            ]
            for si, (src, oidx) in enumerate(srcs):
                gacc = st.tile([P, nblk3 * EMB], f32, name="gacc", tag="gacc")
                for b in range(nblk3):
                    nc.gpsimd.indirect_dma_start(
                        out=gacc[:, b * EMB : (b + 1) * EMB],
                        out_offset=None,
                        in_=src,
                        in_offset=bass.IndirectOffsetOnAxis(
                            ap=oidx[:, b : b + 1], axis=0
                        ),
                    )
                if si == 0:
                    nc.vector.tensor_add(out=acc[:], in0=l3stage[:], in1=gacc[:])
                else:
                    nc.vector.tensor_add(out=acc[:], in0=acc[:], in1=gacc[:])
            nc.vector.tensor_scalar_mul(acc[:], acc[:], 1.0 / (N_LAYERS + 1))
            nc.sync.dma_start(
                out=outbuf[:, :].rearrange("(b p) d -> p b d", p=P),
                in_=acc[:].rearrange("p (b d) -> p b d", d=EMB),
            )
    nc.compile()
    return nc


def _prepare(user_emb, item_emb, adj_vals, adj_rows, adj_cols, users, items):
    ego0 = np.concatenate(
        [np.asarray(user_emb, np.float32), np.asarray(item_emb, np.float32)], axis=0
    )
    adj_rows = np.asarray(adj_rows, np.int64)
    adj_cols = np.asarray(adj_cols, np.int64)
    adj_vals = np.asarray(adj_vals, np.float32)
    users = np.asarray(users, np.int64)
    items = np.asarray(items, np.int64)

    order = np.argsort(adj_rows, kind="stable")
    rows_s, cols_s, vals_s = adj_rows[order], adj_cols[order], adj_vals[order]
    core_bounds = np.searchsorted(rows_s, np.arange(N_CORES + 1) * SHARD)

    deg_all = np.bincount(adj_rows, minlength=N_NODES)
    maxdeg = int(deg_all.max()) if deg_all.size else 0
    p_ch = max(P_CH_MIN, (maxdeg + P - 1) // P)
    p_ch3 = p_ch

    out_nodes = np.unique(np.concatenate([users, USER_COUNT + items]))
    out_owner = out_nodes // SHARD

    # pass 1: per-core block structures
    core_blocks, core_blocks3, core_onodes = [], [], []
    for c in range(N_CORES):
        degs = deg_all[c * SHARD : (c + 1) * SHARD]
        core_blocks.append(_bin_pack(np.arange(SHARD), degs, p_ch * P))
        onodes = out_nodes[out_owner == c]
        odegs = deg_all[onodes]
        core_blocks3.append(_bin_pack(np.arange(len(onodes)), odegs, p_ch3 * P))
        core_onodes.append(onodes)
    nblk = max(len(b) for b in core_blocks)
    nblk3 = max(1, max(len(b) for b in core_blocks3))
    nslot = nblk * P

    # node id -> slot position in the AllGather'd slot-ordered activation
    node_slot = np.zeros(N_NODES, dtype=np.int64)
    for c in range(N_CORES):
        for b, rws in enumerate(core_blocks[c]):
            rws = np.asarray(rws, dtype=np.int64)
            node_slot[c * SHARD + rws] = c * nslot + b * P + np.arange(len(rws))

    in_maps, slotmap = [], {}
    iota = np.tile(np.arange(P, dtype=np.float32), (P, 1))
    for c in range(N_CORES):
        s, e = core_bounds[c], core_bounds[c + 1]
        degs = deg_all[c * SHARD : (c + 1) * SHARD]
        row_start = np.zeros(SHARD + 1, dtype=np.int64)
        np.cumsum(degs, out=row_start[1:])
        cols1, lr, val = _fill_slots(
            core_blocks[c], degs, row_start, cols_s[s:e], vals_s[s:e], p_ch, nblk
        )
        cols2 = node_slot[cols1]

        # layer 3: rows = owned out nodes; nnz grouped by their position
        onodes = core_onodes[c]
        odegs = deg_all[onodes] if len(onodes) else np.empty(0, np.int64)
        o_l = onodes - c * SHARD
        seg_cols = [cols_s[s:e][row_start[r] : row_start[r + 1]] for r in o_l]
        seg_vals = [vals_s[s:e][row_start[r] : row_start[r + 1]] for r in o_l]
        ocols = np.concatenate(seg_cols) if seg_cols else np.empty(0, np.int64)
        ovals = np.concatenate(seg_vals) if seg_vals else np.empty(0, np.float32)
        orow_start = np.zeros(len(onodes) + 1, dtype=np.int64)
        if len(onodes):
            np.cumsum(odegs, out=orow_start[1:])
        cols3n, lr3, val3 = _fill_slots(
            core_blocks3[c], odegs, orow_start, ocols, ovals, p_ch3, nblk3
        )
        cols3 = node_slot[cols3n]

        outrow_nat = np.zeros((P, nblk3), dtype=np.int64)
        for b, opos_list in enumerate(core_blocks3[c]):
            for li, opos in enumerate(opos_list):
                g = int(onodes[opos])
                outrow_nat[li, b] = g
                slotmap[g] = (c, b * P + li)
        outrow_slot = node_slot[outrow_nat]

        in_maps.append(
            {
                "ego0": ego0,
                "cols1": cols1.astype(np.int32),
                "cols2": cols2.astype(np.int32),
                "lr": lr,
                "val": val,
                "cols3": cols3.astype(np.int32),
                "lr3": lr3,
                "val3": val3,
                "outrow_nat": outrow_nat.astype(np.int32),
                "outrow_slot": outrow_slot.astype(np.int32),
                "iota": iota,
            }
        )
    return in_maps, slotmap, nblk, p_ch, nblk3, p_ch3, users, items


_NC_CACHE = {}


def kernel(user_emb, item_emb, adj_vals, adj_rows, adj_cols, users, items,
           _trace=False):
    in_maps, slotmap, nblk, p_ch, nblk3, p_ch3, users, items = _prepare(
        user_emb, item_emb, adj_vals, adj_rows, adj_cols, users, items
    )
    key = (nblk, p_ch, nblk3, p_ch3)
    if key not in _NC_CACHE:
        _NC_CACHE[key] = _build_nc(*key)
    nc = _NC_CACHE[key]
    res = run_bass_kernel_spmd(
        nc, in_maps, core_ids=list(range(N_CORES)), trace=_trace
    )
    outs = [res.results[c]["outbuf"] for c in range(N_CORES)]
    if _trace:
        kernel.last_exec_time_ns = res.exec_time_ns

    user_out = np.empty((len(users), EMB), dtype=np.float32)
    item_out = np.empty((len(items), EMB), dtype=np.float32)
    for i, u in enumerate(users):
        cc, sl = slotmap[int(u)]
        user_out[i] = outs[cc][sl]
    for i, it in enumerate(items):
        cc, sl = slotmap[int(USER_COUNT + it)]
        item_out[i] = outs[cc][sl]
    return user_out, item_out
